# revision 25
# baseline (speedup 1.0000x reference)
"""Trainium2 Bass kernel for nn_DemandExtraction (sparse_attention).

Sharding: candidate pool (n_items=8192) is sharded 1024-per-core across 8
NeuronCores; the small session branch (B=16, L=50) plus all projection
weights are replicated on every core. Each core gathers+scores its slice of
the candidate pool; core 0's copy of the session outputs is used.

Self-contained: hardcodes all shapes; imports only the installed concourse
runtime.
"""

import os
import sys

for _p in ("/opt/trn_rl_repo", "/root/.axon_site/_ro/trn_rl_repo"):
    if os.path.isdir(_p) and _p not in sys.path:
        sys.path.insert(0, _p)

import numpy as np

import concourse.bacc as bacc
import concourse.bass as bass
import concourse.tile as tile
from concourse import mybir
from concourse.bass_utils import run_bass_kernel_spmd
from concourse.masks import make_identity

F32 = mybir.dt.float32
F32R = mybir.dt.float32r
BF16 = mybir.dt.bfloat16
I32 = mybir.dt.int32
AF = mybir.ActivationFunctionType
ALU = mybir.AluOpType

# Problem constants
B, L, D, H, EMB, NCAT, NITEMS = 16, 50, 8, 128, 128, 1000, 8192
NCORES = 8
NC_PER = NITEMS // NCORES  # 1024 candidates per core
SESS = B * L  # 800
SESS_TILES = 7  # ceil(800/128)
CAND_TILES = NC_PER // 128  # 8

# Tuning knobs (env-overridable for experiments; defaults are shipped values)
SCORE_DT = os.environ.get("K_SCORE_DT", "f32")  # f32 | f32r | bf16
U_DT = os.environ.get("K_U_DT", "f32")  # f32 | bf16 (relu-input precision)
VEC_SHARE = int(os.environ.get("K_VEC_SHARE", "10"))  # of 16 relu ops on DVE
S_ENGINE = os.environ.get("K_S_ENGINE", "gpsimd")  # gpsimd | vector

_SCORE_DTYPES = {"f32": F32, "f32r": F32R, "bf16": BF16, "f16": mybir.dt.float16}

_CACHE = {}


def _emit(nc):
    emb_table = nc.declare_dram_parameter("emb_table", [NCAT, EMB], F32, False)
    cand_idx = nc.declare_dram_parameter("cand_idx", [128, CAND_TILES], I32, False)
    sess_idx = nc.declare_dram_parameter("sess_idx", [128, SESS_TILES], I32, False)
    Wk = nc.declare_dram_parameter("Wk", [EMB, H], F32, False)
    Wd = nc.declare_dram_parameter("Wd", [EMB, D * H], F32, False)
    Wd1 = nc.declare_dram_parameter("Wd1", [H, H], F32, False)
    Wk1 = nc.declare_dram_parameter("Wk1", [H, H], F32, False)
    Wc = nc.declare_dram_parameter("Wc", [H, NCAT], F32, False)
    bk = nc.declare_dram_parameter("bk", [H, 1], F32, False)
    b1 = nc.declare_dram_parameter("b1", [H, 1], F32, False)
    wsc = nc.declare_dram_parameter("wsc", [H, 1], F32, False)
    bc = nc.declare_dram_parameter("bc", [1, NCAT], F32, False)

    emb_cand_out = nc.declare_dram_parameter("emb_cand_out", [NC_PER, EMB], F32, True)
    # transposed: rows = candidate index within this core's slice, cols = b*D+d
    dsc_out = nc.declare_dram_parameter("dsc_out", [NC_PER, B * D], F32, True)
    emb_out = nc.declare_dram_parameter("emb_out", [SESS, EMB], F32, True)
    t2_out = nc.declare_dram_parameter("t2_out", [128, 50], F32, True)
    aggT_out = nc.declare_dram_parameter("aggT_out", [H, D * B], F32, True)
    catgy_out = nc.declare_dram_parameter("catgy_out", [D * B, NCAT], F32, True)

    with tile.TileContext(nc) as tc:
        with (
            tc.tile_pool(name="const", bufs=1) as const,
            tc.tile_pool(name="work", bufs=2) as work,
            tc.tile_pool(name="gpool", bufs=3) as gpool,
            tc.tile_pool(name="expp", bufs=2) as expp,
            tc.tile_pool(name="vpool", bufs=4) as vpool,
            tc.tile_pool(name="ps_tr", bufs=2, space="PSUM") as ps_tr,
            tc.tile_pool(name="ps_proj", bufs=1, space="PSUM") as ps_proj,
            tc.tile_pool(name="ps_hd", bufs=1, space="PSUM") as ps_hd,
            tc.tile_pool(name="ps_dsc", bufs=1, space="PSUM") as ps_dsc,
        ):
            # ---- constants into SBUF
            ident = const.tile([128, 128], F32)
            make_identity(nc, ident[:])
            Wk_s = const.tile([EMB, H], F32)
            nc.sync.dma_start(out=Wk_s[:], in_=Wk[:])
            Wd_s = const.tile([EMB, D * H], F32)
            nc.sync.dma_start(out=Wd_s[:], in_=Wd[:])
            Wd1_s = const.tile([H, H], F32)
            nc.sync.dma_start(out=Wd1_s[:], in_=Wd1[:])
            Wk1_s = const.tile([H, H], F32)
            nc.sync.dma_start(out=Wk1_s[:], in_=Wk1[:])
            Wc_s = const.tile([H, NCAT], F32)
            nc.sync.dma_start(out=Wc_s[:], in_=Wc[:])
            bk_s = const.tile([H, 1], F32)
            nc.sync.dma_start(out=bk_s[:], in_=bk[:])
            b1_s = const.tile([H, 1], F32)
            nc.sync.dma_start(out=b1_s[:], in_=b1[:])
            wsc_s = const.tile([H, 1], F32)
            nc.sync.dma_start(out=wsc_s[:], in_=wsc[:])
            bc_s = const.tile([1, NCAT], F32)
            nc.sync.dma_start(out=bc_s[:], in_=bc[:])
            ones1 = const.tile([1, 128], F32)
            nc.vector.memset(ones1[:], 1.0)
            cand_s = const.tile([128, CAND_TILES], I32)
            nc.sync.dma_start(out=cand_s[:], in_=cand_idx[:])
            sess_s = const.tile([128, SESS_TILES], I32)
            nc.sync.dma_start(out=sess_s[:], in_=sess_idx[:])

            embcT = const.tile([EMB, NC_PER], F32)  # emb_cand^T
            embT = const.tile([EMB, SESS_TILES * 128], F32)  # emb^T (800 valid)

            # ---- candidate gathers + transposes
            for j in range(CAND_TILES):
                g = gpool.tile([128, EMB], F32, tag="gath")
                nc.gpsimd.indirect_dma_start(
                    out=g[:],
                    out_offset=None,
                    in_=emb_table[:],
                    in_offset=bass.IndirectOffsetOnAxis(ap=cand_s[:, j : j + 1], axis=0),
                )
                nc.sync.dma_start(out=emb_cand_out[j * 128 : (j + 1) * 128, :], in_=g[:])
                pt = ps_tr.tile([128, 128], F32, tag="ps_tr")
                nc.tensor.transpose(out=pt[:], in_=g[:], identity=ident[:])
                nc.scalar.copy(out=embcT[:, j * 128 : (j + 1) * 128], in_=pt[:])

            # ---- session gathers + transposes
            for j in range(SESS_TILES):
                g = gpool.tile([128, EMB], F32, tag="gath")
                nc.gpsimd.indirect_dma_start(
                    out=g[:],
                    out_offset=None,
                    in_=emb_table[:],
                    in_offset=bass.IndirectOffsetOnAxis(ap=sess_s[:, j : j + 1], axis=0),
                )
                if j < 6:
                    nc.sync.dma_start(out=emb_out[j * 128 : (j + 1) * 128, :], in_=g[:])
                else:
                    nc.sync.dma_start(out=emb_out[768:800, :], in_=g[:32, :])
                pt = ps_tr.tile([128, 128], F32, tag="ps_tr")
                nc.tensor.transpose(out=pt[:], in_=g[:], identity=ident[:])
                nc.vector.tensor_copy(out=embT[:, j * 128 : (j + 1) * 128], in_=pt[:])

            # ---- candidate projections: U = Wk1^T (Wk^T embcT + bk) + b1
            hkc_ps = ps_proj.tile([128, NC_PER], F32, tag="ps_proj")
            nc.tensor.matmul(out=hkc_ps[:, :512], lhsT=Wk_s[:], rhs=embcT[:, :512], start=True, stop=True)
            nc.tensor.matmul(out=hkc_ps[:, 512:], lhsT=Wk_s[:], rhs=embcT[:, 512:], start=True, stop=True)
            hkcT = work.tile([128, NC_PER], F32, tag="hkcT")
            nc.scalar.add(out=hkcT[:], in_=hkc_ps[:], add=bk_s[:, 0:1])
            pkc_ps = ps_proj.tile([128, NC_PER], F32, tag="ps_proj")
            nc.tensor.matmul(out=pkc_ps[:, :512], lhsT=Wk1_s[:], rhs=hkcT[:, :512], start=True, stop=True)
            nc.tensor.matmul(out=pkc_ps[:, 512:], lhsT=Wk1_s[:], rhs=hkcT[:, 512:], start=True, stop=True)
            u_dt = _SCORE_DTYPES.get(U_DT, F32) if U_DT != "f32r" else F32
            U = const.tile([128, NC_PER], u_dt)
            nc.scalar.add(out=U[:], in_=pkc_ps[:], add=b1_s[:, 0:1])

            # ---- session projections: pkbT = Wk1^T (Wk^T embT + bk) + b1
            hk_ps = ps_proj.tile([128, SESS], F32, tag="ps_proj")
            nc.tensor.matmul(out=hk_ps[:, :512], lhsT=Wk_s[:], rhs=embT[:, :512], start=True, stop=True)
            nc.tensor.matmul(out=hk_ps[:, 512:], lhsT=Wk_s[:], rhs=embT[:, 512:SESS], start=True, stop=True)
            hkT = work.tile([128, SESS], F32, tag="hkT")
            nc.scalar.add(out=hkT[:], in_=hk_ps[:], add=bk_s[:, 0:1])
            pk_ps = ps_proj.tile([128, SESS], F32, tag="ps_proj")
            nc.tensor.matmul(out=pk_ps[:, :512], lhsT=Wk1_s[:], rhs=hkT[:, :512], start=True, stop=True)
            nc.tensor.matmul(out=pk_ps[:, 512:], lhsT=Wk1_s[:], rhs=hkT[:, 512:], start=True, stop=True)
            pkbT = const.tile([128, SESS], F32)
            nc.scalar.add(out=pkbT[:], in_=pk_ps[:], add=b1_s[:, 0:1])

            # ---- hidden_demand -> logsumexp over L (values are tiny: direct
            # exp-sum-log is numerically safe here)
            sums = const.tile([128, D * B], F32)
            for j in range(D):
                hd_ps = ps_hd.tile([128, SESS], F32, tag="ps_hd")
                wslice = Wd_s[:, j * 128 : (j + 1) * 128]
                nc.tensor.matmul(out=hd_ps[:, :512], lhsT=wslice, rhs=embT[:, :512], start=True, stop=True)
                nc.tensor.matmul(out=hd_ps[:, 512:], lhsT=wslice, rhs=embT[:, 512:SESS], start=True, stop=True)
                ex = expp.tile([128, SESS], F32, tag="exp")
                nc.scalar.activation(out=ex[:], in_=hd_ps[:], func=AF.Exp)
                nc.vector.tensor_reduce(
                    out=sums[:, j * B : (j + 1) * B],
                    in_=ex[:].rearrange("p (b l) -> p b l", l=L),
                    axis=mybir.AxisListType.X,
                    op=ALU.add,
                )
            aggT = const.tile([128, D * B], F32)  # [h, d*16+b]
            nc.scalar.activation(out=aggT[:], in_=sums[:], func=AF.Ln)
            nc.sync.dma_start(out=aggT_out[:], in_=aggT[:])

            # ---- pdT = Wd1^T @ aggT  [h, d*16+b]
            pd_ps = ps_tr.tile([128, 128], F32, tag="ps_tr")
            nc.tensor.matmul(out=pd_ps[:], lhsT=Wd1_s[:], rhs=aggT[:], start=True, stop=True)
            pdT = const.tile([128, D * B], F32)
            nc.scalar.copy(out=pdT[:], in_=pd_ps[:])

            # ---- catgy_score^T-ish: rows d*16+b, cols categories
            cat_ps = ps_proj.tile([128, NCAT], F32, tag="ps_proj")
            nc.tensor.matmul(out=cat_ps[:, :512], lhsT=aggT[:], rhs=Wc_s[:, :512], start=True, stop=False)
            nc.tensor.matmul(out=cat_ps[:, :512], lhsT=ones1[:], rhs=bc_s[:, :512], start=False, stop=True)
            nc.tensor.matmul(out=cat_ps[:, 512:], lhsT=aggT[:], rhs=Wc_s[:, 512:], start=True, stop=False)
            nc.tensor.matmul(out=cat_ps[:, 512:], lhsT=ones1[:], rhs=bc_s[:, 512:], start=False, stop=True)
            cat_sb = work.tile([128, NCAT], F32, tag="cat_sb")
            nc.scalar.copy(out=cat_sb[:], in_=cat_ps[:])
            nc.sync.dma_start(out=catgy_out[:], in_=cat_sb[:])

            # ---- demand_score: S[h,(b,d,l)] = relu(pkb[h,(b,l)] + pd[h,(d,b)])
            mm_dt = _SCORE_DTYPES[SCORE_DT]
            S = const.tile([128, B, D, L], mm_dt)
            pkb_ap = bass.AP(
                tensor=pkbT[:].tensor,
                offset=pkbT[:].offset,
                ap=[pkbT[:].ap[0], [L, B], [0, D], [1, L]],
            )
            pd_ap = bass.AP(
                tensor=pdT[:].tensor,
                offset=pdT[:].offset,
                ap=[pdT[:].ap[0], [1, B], [B, D], [0, L]],
            )
            s_eng = nc.gpsimd if S_ENGINE == "gpsimd" else nc.vector
            s_eng.tensor_tensor(out=S[:], in0=pkb_ap, in1=pd_ap, op=ALU.add)
            s_eng.tensor_scalar_max(S[:], S[:], 0.0)
            if SCORE_DT == "f32":
                wsc_mm = wsc_s
            else:
                wsc_mm = const.tile([H, 1], mm_dt)
                nc.vector.tensor_copy(out=wsc_mm[:], in_=wsc_s[:])
            Sf = S[:].rearrange("p b d l -> p (b d l)")
            t2_ps = ps_tr.tile([128, 64], F32, tag="ps_tr")
            for m in range(50):
                nc.tensor.matmul(
                    out=t2_ps[:, m : m + 1],
                    lhsT=Sf[:, m * 128 : (m + 1) * 128],
                    rhs=wsc_mm[:],
                    start=True,
                    stop=True,
                )
            t2_sb = work.tile([128, 64], F32, tag="t2_sb")
            nc.vector.tensor_copy(out=t2_sb[:, :50], in_=t2_ps[:, :50])
            nc.sync.dma_start(out=t2_out[:], in_=t2_sb[:, :50])

            # ---- main loop: per (b,d) pair r, relu-tile V_r = relu(U + pd_r),
            # then 8 matmuls with lhsT = 128-candidate chunk of V_r (weights)
            # and rhs = w_score: each lands a (128,1) column at pall[:, c*128+r].
            # one 2-bank psum tile holds all chunk outputs: column = c*128 + r
            pall = ps_dsc.tile([128, CAND_TILES * B * D], F32, tag="dsc")
            for r in range(B * D):
                b_, d_ = r // D, r % D
                col = d_ * B + b_
                V = vpool.tile([128, NC_PER], mm_dt, tag="V")
                if (r * VEC_SHARE) % 16 < VEC_SHARE:
                    nc.vector.tensor_scalar(
                        V[:], U[:], pdT[:, col : col + 1], 0.0, ALU.add, ALU.max
                    )
                else:
                    nc.scalar.activation(
                        out=V[:], in_=U[:], func=AF.Relu, bias=pdT[:, col : col + 1]
                    )
                for c in range(CAND_TILES):
                    cr = c * B * D + r
                    nc.tensor.matmul(
                        out=pall[:, cr : cr + 1],
                        lhsT=V[:, c * 128 : (c + 1) * 128],
                        rhs=wsc_mm[:],
                        start=True, stop=True,
                    )
            dsb = work.tile([128, CAND_TILES * B * D], F32, tag="dsb")
            nc.scalar.copy(out=dsb[:, :512], in_=pall[:, :512])
            nc.scalar.copy(out=dsb[:, 512:], in_=pall[:, 512:])
            for c in range(CAND_TILES):
                nc.sync.dma_start(
                    out=dsc_out[c * 128 : (c + 1) * 128, :],
                    in_=dsb[:, c * B * D : (c + 1) * B * D],
                )


def _build():
    key = (SCORE_DT, U_DT, VEC_SHARE, S_ENGINE)
    if key in _CACHE:
        return _CACHE[key]
    nc = bacc.Bacc("TRN2", target_bir_lowering=False, debug=False, num_devices=NCORES)
    _emit(nc)
    nc.compile()
    _CACHE[key] = nc
    return nc


def kernel(**inputs):
    nc = _build()

    input_ids = np.asarray(inputs["input"]).astype(np.int32)  # (B, L)
    cand = np.asarray(inputs["candidate_pool_category"]).astype(np.int32)  # (NITEMS,)
    emb_table = np.ascontiguousarray(np.asarray(inputs["emb_table"], dtype=np.float32))
    Wd = np.ascontiguousarray(np.asarray(inputs["Wd"], dtype=np.float32))
    Wk = np.ascontiguousarray(np.asarray(inputs["Wk"], dtype=np.float32))
    bk = np.asarray(inputs["bk"], dtype=np.float32).reshape(H, 1)
    W1 = np.asarray(inputs["W1"], dtype=np.float32)
    Wd1 = np.ascontiguousarray(W1[:H])
    Wk1 = np.ascontiguousarray(W1[H:])
    b1 = np.asarray(inputs["b1"], dtype=np.float32).reshape(H, 1)
    wsc = np.asarray(inputs["w_score"], dtype=np.float32).reshape(H, 1)
    Wc = np.ascontiguousarray(np.asarray(inputs["Wc"], dtype=np.float32))
    bc = np.asarray(inputs["bc"], dtype=np.float32).reshape(1, NCAT)

    sess_pad = np.zeros(SESS_TILES * 128, np.int32)
    sess_pad[:SESS] = input_ids.reshape(-1)
    sess_2d = np.ascontiguousarray(sess_pad.reshape(SESS_TILES, 128).T)

    shared = dict(
        emb_table=emb_table, sess_idx=sess_2d, Wk=Wk, Wd=Wd, Wd1=Wd1, Wk1=Wk1,
        Wc=Wc, bk=bk, b1=b1, wsc=wsc, bc=bc,
    )
    in_maps = []
    for c in range(NCORES):
        sl = cand[c * NC_PER : (c + 1) * NC_PER]
        cand_2d = np.ascontiguousarray(sl.reshape(CAND_TILES, 128).T)
        m = dict(shared)
        m["cand_idx"] = cand_2d
        in_maps.append(m)

    trace = bool(int(os.environ.get("K_TRACE", "0")))
    res = run_bass_kernel_spmd(nc, in_maps, list(range(NCORES)), trace=trace)
    kernel.last_results = res

    r0 = res.results[0]
    emb = r0["emb_out"].reshape(B, L, EMB)
    emb_cand = np.concatenate(
        [res.results[c]["emb_cand_out"] for c in range(NCORES)], axis=0
    )
    # per-core dsc_out is (NC_PER, B*D); stack along n then transpose
    dscT = np.concatenate(
        [res.results[c]["dsc_out"] for c in range(NCORES)], axis=0
    )  # (NITEMS, B*D)
    dsc = np.ascontiguousarray(dscT.T).reshape(B, D, NITEMS)

    aggT = r0["aggT_out"]  # [h, d*16+b]
    agg = aggT.T.reshape(D, B, H).transpose(1, 0, 2)  # (B, D, H)
    catgy_score = r0["catgy_out"].reshape(D, B, NCAT).transpose(1, 0, 2)

    # t2_out[c, m] = score of flat (b*D*L + d*L + l) index m*128+c
    t2 = r0["t2_out"]
    t2bdl = np.ascontiguousarray(t2.T).reshape(-1)[: B * D * L].reshape(B, D, L)
    # faithful torch .view(B, D, L) of a contiguous (B, L, D) tensor
    demand_score = t2bdl.transpose(0, 2, 1).reshape(B, D, L)

    hn = agg / (np.linalg.norm(agg, axis=-1, keepdims=True) + 1e-12)
    sim = np.einsum("bdh,beh->bde", hn, hn)
    off = sim * (1.0 - np.eye(D, dtype=sim.dtype))
    demand_sim_loss = np.float32(off.sum() / (B * D * (D - 1)))

    return (catgy_score, demand_score, dsc, emb, emb_cand, demand_sim_loss)


# revision 40
# speedup vs baseline: 1.3705x; 1.3705x over previous
"""Trainium2 Bass kernel for nn_DemandExtraction (sparse_attention).

Sharding: candidate pool (n_items=8192) is sharded 1024-per-core across 8
NeuronCores; the small session branch (B=16, L=50) plus all projection
weights are replicated on every core. Each core gathers+scores its slice of
the candidate pool; core 0's copy of the session outputs is used.

Self-contained: hardcodes all shapes; imports only the installed concourse
runtime.
"""

import os
import sys

for _p in ("/opt/trn_rl_repo", "/root/.axon_site/_ro/trn_rl_repo"):
    if os.path.isdir(_p) and _p not in sys.path:
        sys.path.insert(0, _p)

import numpy as np

import concourse.bacc as bacc
import concourse.bass as bass
import concourse.tile as tile
from concourse import mybir
from concourse.bass_utils import run_bass_kernel_spmd
from concourse.masks import make_identity

F32 = mybir.dt.float32
F32R = mybir.dt.float32r
BF16 = mybir.dt.bfloat16
I32 = mybir.dt.int32
AF = mybir.ActivationFunctionType
ALU = mybir.AluOpType

# Problem constants
B, L, D, H, EMB, NCAT, NITEMS = 16, 50, 8, 128, 128, 1000, 8192
NCORES = 8
NC_PER = NITEMS // NCORES  # 1024 candidates per core
SESS = B * L  # 800
SESS_TILES = 7  # ceil(800/128)
CAND_TILES = NC_PER // 128  # 8

# Tuning knobs (env-overridable for experiments; defaults are shipped values)
SCORE_DT = os.environ.get("K_SCORE_DT", "f16")  # f32 | f32r | bf16 | f16
U_DT = os.environ.get("K_U_DT", "f16")  # f32 | bf16 | f16 (relu-input precision)
VEC_SHARE = int(os.environ.get("K_VEC_SHARE", "13"))  # of 16 relu ops on DVE
S_ENGINE = os.environ.get("K_S_ENGINE", "vector")  # gpsimd | vector

_SCORE_DTYPES = {"f32": F32, "f32r": F32R, "bf16": BF16, "f16": mybir.dt.float16}

_CACHE = {}


def _emit(nc):
    emb_table = nc.declare_dram_parameter("emb_table", [NCAT, EMB], F32, False)
    cand_idx = nc.declare_dram_parameter("cand_idx", [128, CAND_TILES], I32, False)
    sess_idx = nc.declare_dram_parameter("sess_idx", [128, SESS_TILES], I32, False)
    Wk = nc.declare_dram_parameter("Wk", [EMB, H], F32, False)
    Wd = nc.declare_dram_parameter("Wd", [EMB, D * H], F32, False)
    Wd1 = nc.declare_dram_parameter("Wd1", [H, H], F32, False)
    Wk1 = nc.declare_dram_parameter("Wk1", [H, H], F32, False)
    Wc = nc.declare_dram_parameter("Wc", [H, NCAT], F32, False)
    bk = nc.declare_dram_parameter("bk", [H, 1], F32, False)
    b1 = nc.declare_dram_parameter("b1", [H, 1], F32, False)
    wsc = nc.declare_dram_parameter("wsc", [H, 1], F32, False)
    bc = nc.declare_dram_parameter("bc", [1, NCAT], F32, False)

    emb_cand_out = nc.declare_dram_parameter("emb_cand_out", [NC_PER, EMB], F32, True)
    # rows = candidate-within-chunk n', cols = r*8+c (r = b*D+d, c = chunk)
    dsc_out = nc.declare_dram_parameter("dsc_out", [128, CAND_TILES * B * D], F32, True)
    emb_out = nc.declare_dram_parameter("emb_out", [SESS, EMB], F32, True)
    t2_out = nc.declare_dram_parameter("t2_out", [128, 50], F32, True)
    aggT_out = nc.declare_dram_parameter("aggT_out", [H, D * B], F32, True)
    catgy_out = nc.declare_dram_parameter("catgy_out", [D * B, NCAT], F32, True)

    with tile.TileContext(nc) as tc:
        with (
            tc.tile_pool(name="const", bufs=1) as const,
            tc.tile_pool(name="work", bufs=2) as work,
            tc.tile_pool(name="gpool", bufs=4) as gpool,
            tc.tile_pool(name="expp", bufs=2) as expp,
            tc.tile_pool(name="vpool", bufs=8) as vpool,
            tc.tile_pool(name="ps_tr", bufs=2, space="PSUM") as ps_tr,
            tc.tile_pool(name="ps_proj", bufs=1, space="PSUM") as ps_proj,
            tc.tile_pool(name="ps_hd", bufs=1, space="PSUM") as ps_hd,
            tc.tile_pool(name="ps_dsc", bufs=1, space="PSUM") as ps_dsc,
        ):
            # ---- constants into SBUF
            ident = const.tile([128, 128], F32)
            make_identity(nc, ident[:])
            Wk_s = const.tile([EMB, H], F32)
            nc.sync.dma_start(out=Wk_s[:], in_=Wk[:])
            Wd_s = const.tile([EMB, D * H], F32)
            nc.sync.dma_start(out=Wd_s[:], in_=Wd[:])
            Wd1_s = const.tile([H, H], F32)
            nc.sync.dma_start(out=Wd1_s[:], in_=Wd1[:])
            Wk1_s = const.tile([H, H], F32)
            nc.sync.dma_start(out=Wk1_s[:], in_=Wk1[:])
            Wc_s = const.tile([H, NCAT], F32)
            nc.sync.dma_start(out=Wc_s[:], in_=Wc[:])
            bk_s = const.tile([H, 1], F32)
            nc.sync.dma_start(out=bk_s[:], in_=bk[:])
            b1_s = const.tile([H, 1], F32)
            nc.sync.dma_start(out=b1_s[:], in_=b1[:])
            wsc_s = const.tile([H, 1], F32)
            nc.sync.dma_start(out=wsc_s[:], in_=wsc[:])
            bc_s = const.tile([1, NCAT], F32)
            nc.sync.dma_start(out=bc_s[:], in_=bc[:])
            ones1 = const.tile([1, 128], F32)
            nc.vector.memset(ones1[:], 1.0)
            cand_s = const.tile([128, CAND_TILES], I32)
            nc.sync.dma_start(out=cand_s[:], in_=cand_idx[:])
            sess_s = const.tile([128, SESS_TILES], I32)
            nc.sync.dma_start(out=sess_s[:], in_=sess_idx[:])

            embcT = const.tile([EMB, NC_PER], F32)  # emb_cand^T
            embT = const.tile([EMB, SESS_TILES * 128], F32)  # emb^T (800 valid)

            # ---- session gathers + transposes first: the session chain
            # (agg -> pdT) is the longest dependency path to the main loop.
            # Per-tile (P,1) offsets: the only indirect-DMA shape walrus
            # lowers correctly on HW.
            for j in range(SESS_TILES):
                g = gpool.tile([128, EMB], F32, tag="gath")
                nc.gpsimd.indirect_dma_start(
                    out=g[:],
                    out_offset=None,
                    in_=emb_table[:],
                    in_offset=bass.IndirectOffsetOnAxis(ap=sess_s[:, j : j + 1], axis=0),
                )
                if j < 6:
                    nc.sync.dma_start(out=emb_out[j * 128 : (j + 1) * 128, :], in_=g[:])
                else:
                    nc.sync.dma_start(out=emb_out[768:800, :], in_=g[:32, :])
                pt = ps_tr.tile([128, 128], F32, tag="ps_tr")
                nc.tensor.transpose(out=pt[:], in_=g[:], identity=ident[:])
                nc.vector.tensor_copy(out=embT[:, j * 128 : (j + 1) * 128], in_=pt[:])

            # ---- candidate gathers + transposes
            for j in range(CAND_TILES):
                g = gpool.tile([128, EMB], F32, tag="gath")
                nc.gpsimd.indirect_dma_start(
                    out=g[:],
                    out_offset=None,
                    in_=emb_table[:],
                    in_offset=bass.IndirectOffsetOnAxis(ap=cand_s[:, j : j + 1], axis=0),
                )
                nc.sync.dma_start(out=emb_cand_out[j * 128 : (j + 1) * 128, :], in_=g[:])
                pt = ps_tr.tile([128, 128], F32, tag="ps_tr")
                nc.tensor.transpose(out=pt[:], in_=g[:], identity=ident[:])
                nc.scalar.copy(out=embcT[:, j * 128 : (j + 1) * 128], in_=pt[:])

            # ---- session projections: pkbT = Wk1^T (Wk^T embT + bk) + b1
            hk_ps = ps_proj.tile([128, SESS], F32, tag="ps_proj")
            nc.tensor.matmul(out=hk_ps[:, :512], lhsT=Wk_s[:], rhs=embT[:, :512], start=True, stop=True)
            nc.tensor.matmul(out=hk_ps[:, 512:], lhsT=Wk_s[:], rhs=embT[:, 512:SESS], start=True, stop=True)
            hkT = work.tile([128, SESS], F32, tag="hkT")
            nc.scalar.add(out=hkT[:], in_=hk_ps[:], add=bk_s[:, 0:1])
            pk_ps = ps_proj.tile([128, SESS], F32, tag="ps_proj")
            nc.tensor.matmul(out=pk_ps[:, :512], lhsT=Wk1_s[:], rhs=hkT[:, :512], start=True, stop=True)
            nc.tensor.matmul(out=pk_ps[:, 512:], lhsT=Wk1_s[:], rhs=hkT[:, 512:], start=True, stop=True)
            pkbT = const.tile([128, SESS], F32)
            nc.scalar.add(out=pkbT[:], in_=pk_ps[:], add=b1_s[:, 0:1])

            # ---- hidden_demand -> logsumexp over L (values are tiny: direct
            # exp-sum-log is numerically safe here)
            sums = const.tile([128, D * B], F32)
            for j in range(D):
                hd_ps = ps_hd.tile([128, SESS], F32, tag="ps_hd")
                wslice = Wd_s[:, j * 128 : (j + 1) * 128]
                nc.tensor.matmul(out=hd_ps[:, :512], lhsT=wslice, rhs=embT[:, :512], start=True, stop=True)
                nc.tensor.matmul(out=hd_ps[:, 512:], lhsT=wslice, rhs=embT[:, 512:SESS], start=True, stop=True)
                ex = expp.tile([128, SESS], F32, tag="exp")
                nc.scalar.activation(out=ex[:], in_=hd_ps[:], func=AF.Exp)
                nc.vector.tensor_reduce(
                    out=sums[:, j * B : (j + 1) * B],
                    in_=ex[:].rearrange("p (b l) -> p b l", l=L),
                    axis=mybir.AxisListType.X,
                    op=ALU.add,
                )
            aggT = const.tile([128, D * B], F32)  # [h, d*16+b]
            nc.scalar.activation(out=aggT[:], in_=sums[:], func=AF.Ln)
            nc.sync.dma_start(out=aggT_out[:], in_=aggT[:])

            # ---- pdT = Wd1^T @ aggT  [h, d*16+b]
            pd_ps = ps_tr.tile([128, 128], F32, tag="ps_tr")
            nc.tensor.matmul(out=pd_ps[:], lhsT=Wd1_s[:], rhs=aggT[:], start=True, stop=True)
            pdT = const.tile([128, D * B], F32)
            nc.scalar.copy(out=pdT[:], in_=pd_ps[:])

            # ---- candidate projections: U = Wk1^T (Wk^T embcT + bk) + b1
            hkc_ps = ps_proj.tile([128, NC_PER], F32, tag="ps_proj")
            nc.tensor.matmul(out=hkc_ps[:, :512], lhsT=Wk_s[:], rhs=embcT[:, :512], start=True, stop=True)
            nc.tensor.matmul(out=hkc_ps[:, 512:], lhsT=Wk_s[:], rhs=embcT[:, 512:], start=True, stop=True)
            hkcT = work.tile([128, NC_PER], F32, tag="hkcT")
            nc.scalar.add(out=hkcT[:], in_=hkc_ps[:], add=bk_s[:, 0:1])
            pkc_ps = ps_proj.tile([128, NC_PER], F32, tag="ps_proj")
            nc.tensor.matmul(out=pkc_ps[:, :512], lhsT=Wk1_s[:], rhs=hkcT[:, :512], start=True, stop=True)
            nc.tensor.matmul(out=pkc_ps[:, 512:], lhsT=Wk1_s[:], rhs=hkcT[:, 512:], start=True, stop=True)
            u_dt = _SCORE_DTYPES.get(U_DT, F32) if U_DT != "f32r" else F32
            U = const.tile([128, NC_PER], u_dt)
            nc.scalar.add(out=U[:], in_=pkc_ps[:], add=b1_s[:, 0:1])

            # ---- catgy_score^T-ish: rows d*16+b, cols categories
            cat_ps = ps_proj.tile([128, NCAT], F32, tag="ps_proj")
            nc.tensor.matmul(out=cat_ps[:, :512], lhsT=aggT[:], rhs=Wc_s[:, :512], start=True, stop=False)
            nc.tensor.matmul(out=cat_ps[:, :512], lhsT=ones1[:], rhs=bc_s[:, :512], start=False, stop=True)
            nc.tensor.matmul(out=cat_ps[:, 512:], lhsT=aggT[:], rhs=Wc_s[:, 512:], start=True, stop=False)
            nc.tensor.matmul(out=cat_ps[:, 512:], lhsT=ones1[:], rhs=bc_s[:, 512:], start=False, stop=True)
            cat_sb = work.tile([128, NCAT], F32, tag="cat_sb")
            nc.scalar.copy(out=cat_sb[:], in_=cat_ps[:])
            nc.sync.dma_start(out=catgy_out[:], in_=cat_sb[:])

            # ---- demand_score: S[h,(b,d,l)] = relu(pkb[h,(b,l)] + pd[h,(d,b)])
            # (stays f32: only 50 small matmuls, keeps demand_score near-exact)
            mm_dt = _SCORE_DTYPES[SCORE_DT]
            S = const.tile([128, B, D, L], F32)
            pkb_ap = bass.AP(
                tensor=pkbT[:].tensor,
                offset=pkbT[:].offset,
                ap=[pkbT[:].ap[0], [L, B], [0, D], [1, L]],
            )
            pd_ap = bass.AP(
                tensor=pdT[:].tensor,
                offset=pdT[:].offset,
                ap=[pdT[:].ap[0], [1, B], [B, D], [0, L]],
            )
            s_eng = nc.gpsimd if S_ENGINE == "gpsimd" else nc.vector
            s_eng.tensor_tensor(out=S[:], in0=pkb_ap, in1=pd_ap, op=ALU.add)
            s_eng.tensor_scalar_max(S[:], S[:], 0.0)
            if SCORE_DT == "f32":
                wsc_mm = wsc_s
            else:
                wsc_mm = const.tile([H, 1], mm_dt)
                nc.vector.tensor_copy(out=wsc_mm[:], in_=wsc_s[:])
            Sf = S[:].rearrange("p b d l -> p (b d l)")
            t2_ps = ps_tr.tile([128, 64], F32, tag="ps_tr")
            for m in range(50):
                nc.tensor.matmul(
                    out=t2_ps[:, m : m + 1],
                    lhsT=Sf[:, m * 128 : (m + 1) * 128],
                    rhs=wsc_s[:],
                    start=True,
                    stop=True,
                )
            t2_sb = work.tile([128, 64], F32, tag="t2_sb")
            nc.vector.tensor_copy(out=t2_sb[:, :50], in_=t2_ps[:, :50])
            nc.sync.dma_start(out=t2_out[:], in_=t2_sb[:, :50])

            # ---- main loop: per (b,d) pair r, relu-tile V_r = relu(U + pd_r),
            # then 8 matmuls with lhsT = 128-candidate chunk of V_r (weights)
            # and rhs = w_score: each lands a (128,1) column at pall[:, r*8+c].
            # Column order r*8+c lets the first psum half drain at mid-loop.
            pall = ps_dsc.tile([128, CAND_TILES * B * D], F32, tag="dsc")
            dsb = work.tile([128, CAND_TILES * B * D], F32, tag="dsb")
            half = CAND_TILES * B * D // 2
            for r in range(B * D):
                b_, d_ = r // D, r % D
                col = d_ * B + b_
                V = vpool.tile([128, NC_PER], mm_dt, tag="V")
                if (r * VEC_SHARE) % 16 < VEC_SHARE:
                    nc.vector.tensor_scalar(
                        V[:], U[:], pdT[:, col : col + 1], 0.0, ALU.add, ALU.max
                    )
                else:
                    nc.scalar.activation(
                        out=V[:], in_=U[:], func=AF.Relu, bias=pdT[:, col : col + 1]
                    )
                for c in range(CAND_TILES):
                    cr = r * CAND_TILES + c
                    nc.tensor.matmul(
                        out=pall[:, cr : cr + 1],
                        lhsT=V[:, c * 128 : (c + 1) * 128],
                        rhs=wsc_mm[:],
                        start=True, stop=True,
                    )
                if r == B * D // 2 - 1:
                    nc.scalar.copy(out=dsb[:, :half], in_=pall[:, :half])
                    nc.sync.dma_start(out=dsc_out[:, :half], in_=dsb[:, :half])
            nc.scalar.copy(out=dsb[:, half:], in_=pall[:, half:])
            nc.sync.dma_start(out=dsc_out[:, half:], in_=dsb[:, half:])


def _build():
    key = (SCORE_DT, U_DT, VEC_SHARE, S_ENGINE)
    if key in _CACHE:
        return _CACHE[key]
    nc = bacc.Bacc("TRN2", target_bir_lowering=False, debug=False, num_devices=NCORES)
    _emit(nc)
    nc.compile()
    _CACHE[key] = nc
    return nc


def kernel(**inputs):
    nc = _build()

    input_ids = np.asarray(inputs["input"]).astype(np.int32)  # (B, L)
    cand = np.asarray(inputs["candidate_pool_category"]).astype(np.int32)  # (NITEMS,)
    emb_table = np.ascontiguousarray(np.asarray(inputs["emb_table"], dtype=np.float32))
    Wd = np.ascontiguousarray(np.asarray(inputs["Wd"], dtype=np.float32))
    Wk = np.ascontiguousarray(np.asarray(inputs["Wk"], dtype=np.float32))
    bk = np.asarray(inputs["bk"], dtype=np.float32).reshape(H, 1)
    W1 = np.asarray(inputs["W1"], dtype=np.float32)
    Wd1 = np.ascontiguousarray(W1[:H])
    Wk1 = np.ascontiguousarray(W1[H:])
    b1 = np.asarray(inputs["b1"], dtype=np.float32).reshape(H, 1)
    wsc = np.asarray(inputs["w_score"], dtype=np.float32).reshape(H, 1)
    Wc = np.ascontiguousarray(np.asarray(inputs["Wc"], dtype=np.float32))
    bc = np.asarray(inputs["bc"], dtype=np.float32).reshape(1, NCAT)

    sess_pad = np.zeros(SESS_TILES * 128, np.int32)
    sess_pad[:SESS] = input_ids.reshape(-1)
    sess_2d = np.ascontiguousarray(sess_pad.reshape(SESS_TILES, 128).T)

    shared = dict(
        emb_table=emb_table, sess_idx=sess_2d, Wk=Wk, Wd=Wd, Wd1=Wd1, Wk1=Wk1,
        Wc=Wc, bk=bk, b1=b1, wsc=wsc, bc=bc,
    )
    in_maps = []
    for c in range(NCORES):
        sl = cand[c * NC_PER : (c + 1) * NC_PER]
        cand_2d = np.ascontiguousarray(sl.reshape(CAND_TILES, 128).T)
        m = dict(shared)
        m["cand_idx"] = cand_2d
        in_maps.append(m)

    trace = bool(int(os.environ.get("K_TRACE", "0")))
    res = run_bass_kernel_spmd(nc, in_maps, list(range(NCORES)), trace=trace)
    kernel.last_results = res

    r0 = res.results[0]
    emb = r0["emb_out"].reshape(B, L, EMB)
    emb_cand = np.concatenate(
        [res.results[c]["emb_cand_out"] for c in range(NCORES)], axis=0
    )
    # per-core dsc_out is (128 n', 1024 cols=(r, c)); decode to (r, n) and
    # stack core slices along n
    dsc_cores = []
    for c in range(NCORES):
        a = res.results[c]["dsc_out"].reshape(128, B * D, CAND_TILES)
        dsc_cores.append(np.transpose(a, (1, 2, 0)).reshape(B * D, NC_PER))
    dsc = np.concatenate(dsc_cores, axis=1).reshape(B, D, NITEMS)

    aggT = r0["aggT_out"]  # [h, d*16+b]
    agg = aggT.T.reshape(D, B, H).transpose(1, 0, 2)  # (B, D, H)
    catgy_score = r0["catgy_out"].reshape(D, B, NCAT).transpose(1, 0, 2)

    # t2_out[c, m] = score of flat (b*D*L + d*L + l) index m*128+c
    t2 = r0["t2_out"]
    t2bdl = np.ascontiguousarray(t2.T).reshape(-1)[: B * D * L].reshape(B, D, L)
    # faithful torch .view(B, D, L) of a contiguous (B, L, D) tensor
    demand_score = t2bdl.transpose(0, 2, 1).reshape(B, D, L)

    hn = agg / (np.linalg.norm(agg, axis=-1, keepdims=True) + 1e-12)
    sim = np.einsum("bdh,beh->bde", hn, hn)
    off = sim * (1.0 - np.eye(D, dtype=sim.dtype))
    demand_sim_loss = np.float32(off.sum() / (B * D * (D - 1)))

    return (catgy_score, demand_score, dsc, emb, emb_cand, demand_sim_loss)


# revision 41
# speedup vs baseline: 1.3745x; 1.0029x over previous
"""Trainium2 Bass kernel for nn_DemandExtraction (sparse_attention).

Sharding: candidate pool (n_items=8192) is sharded 1024-per-core across 8
NeuronCores; the small session branch (B=16, L=50) plus all projection
weights are replicated on every core. Each core gathers+scores its slice of
the candidate pool; core 0's copy of the session outputs is used.

Self-contained: hardcodes all shapes; imports only the installed concourse
runtime.
"""

import os
import sys

for _p in ("/opt/trn_rl_repo", "/root/.axon_site/_ro/trn_rl_repo"):
    if os.path.isdir(_p) and _p not in sys.path:
        sys.path.insert(0, _p)

import numpy as np

import concourse.bacc as bacc
import concourse.bass as bass
import concourse.tile as tile
from concourse import mybir
from concourse.bass_utils import run_bass_kernel_spmd
from concourse.masks import make_identity

F32 = mybir.dt.float32
F32R = mybir.dt.float32r
BF16 = mybir.dt.bfloat16
I32 = mybir.dt.int32
AF = mybir.ActivationFunctionType
ALU = mybir.AluOpType

# Problem constants
B, L, D, H, EMB, NCAT, NITEMS = 16, 50, 8, 128, 128, 1000, 8192
NCORES = 8
NC_PER = NITEMS // NCORES  # 1024 candidates per core
SESS = B * L  # 800
SESS_TILES = 7  # ceil(800/128)
CAND_TILES = NC_PER // 128  # 8

# Tuning knobs (env-overridable for experiments; defaults are shipped values)
SCORE_DT = os.environ.get("K_SCORE_DT", "f16")  # f32 | f32r | bf16 | f16
U_DT = os.environ.get("K_U_DT", "f16")  # f32 | bf16 | f16 (relu-input precision)
VEC_SHARE = int(os.environ.get("K_VEC_SHARE", "13"))  # of 16 relu ops on DVE
S_ENGINE = os.environ.get("K_S_ENGINE", "vector")  # gpsimd | vector

_SCORE_DTYPES = {"f32": F32, "f32r": F32R, "bf16": BF16, "f16": mybir.dt.float16}

_CACHE = {}


def _emit(nc):
    emb_table = nc.declare_dram_parameter("emb_table", [NCAT, EMB], F32, False)
    cand_idx = nc.declare_dram_parameter("cand_idx", [128, CAND_TILES], I32, False)
    sess_idx = nc.declare_dram_parameter("sess_idx", [128, SESS_TILES], I32, False)
    Wk = nc.declare_dram_parameter("Wk", [EMB, H], F32, False)
    Wd = nc.declare_dram_parameter("Wd", [EMB, D * H], F32, False)
    Wd1 = nc.declare_dram_parameter("Wd1", [H, H], F32, False)
    Wk1 = nc.declare_dram_parameter("Wk1", [H, H], F32, False)
    Wc = nc.declare_dram_parameter("Wc", [H, NCAT], F32, False)
    bk = nc.declare_dram_parameter("bk", [H, 1], F32, False)
    b1 = nc.declare_dram_parameter("b1", [H, 1], F32, False)
    wsc = nc.declare_dram_parameter("wsc", [H, 1], F32, False)
    bc = nc.declare_dram_parameter("bc", [1, NCAT], F32, False)

    emb_cand_out = nc.declare_dram_parameter("emb_cand_out", [NC_PER, EMB], F32, True)
    # rows = candidate-within-chunk n', cols = r*8+c (r = b*D+d, c = chunk)
    dsc_out = nc.declare_dram_parameter("dsc_out", [128, CAND_TILES * B * D], F32, True)
    emb_out = nc.declare_dram_parameter("emb_out", [SESS, EMB], F32, True)
    t2_out = nc.declare_dram_parameter("t2_out", [128, 50], F32, True)
    aggT_out = nc.declare_dram_parameter("aggT_out", [H, D * B], F32, True)
    catgy_out = nc.declare_dram_parameter("catgy_out", [D * B, NCAT], F32, True)

    with tile.TileContext(nc) as tc:
        with (
            tc.tile_pool(name="const", bufs=1) as const,
            tc.tile_pool(name="work", bufs=2) as work,
            tc.tile_pool(name="gpool", bufs=4) as gpool,
            tc.tile_pool(name="expp", bufs=2) as expp,
            tc.tile_pool(name="vpool", bufs=8) as vpool,
            tc.tile_pool(name="ps_tr", bufs=2, space="PSUM") as ps_tr,
            tc.tile_pool(name="ps_proj", bufs=1, space="PSUM") as ps_proj,
            tc.tile_pool(name="ps_hd", bufs=1, space="PSUM") as ps_hd,
            tc.tile_pool(name="ps_dsc", bufs=1, space="PSUM") as ps_dsc,
        ):
            # ---- constants into SBUF
            ident = const.tile([128, 128], F32)
            make_identity(nc, ident[:])
            Wk_s = const.tile([EMB, H], F32)
            nc.sync.dma_start(out=Wk_s[:], in_=Wk[:])
            Wd_s = const.tile([EMB, D * H], F32)
            nc.sync.dma_start(out=Wd_s[:], in_=Wd[:])
            Wd1_s = const.tile([H, H], F32)
            nc.sync.dma_start(out=Wd1_s[:], in_=Wd1[:])
            Wk1_s = const.tile([H, H], F32)
            nc.sync.dma_start(out=Wk1_s[:], in_=Wk1[:])
            Wc_s = const.tile([H, NCAT], F32)
            nc.sync.dma_start(out=Wc_s[:], in_=Wc[:])
            bk_s = const.tile([H, 1], F32)
            nc.sync.dma_start(out=bk_s[:], in_=bk[:])
            b1_s = const.tile([H, 1], F32)
            nc.sync.dma_start(out=b1_s[:], in_=b1[:])
            wsc_s = const.tile([H, 1], F32)
            nc.sync.dma_start(out=wsc_s[:], in_=wsc[:])
            bc_s = const.tile([1, NCAT], F32)
            nc.sync.dma_start(out=bc_s[:], in_=bc[:])
            ones1 = const.tile([1, 128], F32)
            nc.vector.memset(ones1[:], 1.0)
            cand_s = const.tile([128, CAND_TILES], I32)
            nc.sync.dma_start(out=cand_s[:], in_=cand_idx[:])
            sess_s = const.tile([128, SESS_TILES], I32)
            nc.sync.dma_start(out=sess_s[:], in_=sess_idx[:])

            embcT = const.tile([EMB, NC_PER], F32)  # emb_cand^T
            embT = const.tile([EMB, SESS_TILES * 128], F32)  # emb^T (800 valid)

            # ---- session gathers + transposes first: the session chain
            # (agg -> pdT) is the longest dependency path to the main loop.
            # Per-tile (P,1) offsets: the only indirect-DMA shape walrus
            # lowers correctly on HW.
            for j in range(SESS_TILES):
                g = gpool.tile([128, EMB], F32, tag="gath")
                nc.gpsimd.indirect_dma_start(
                    out=g[:],
                    out_offset=None,
                    in_=emb_table[:],
                    in_offset=bass.IndirectOffsetOnAxis(ap=sess_s[:, j : j + 1], axis=0),
                )
                if j < 6:
                    nc.sync.dma_start(out=emb_out[j * 128 : (j + 1) * 128, :], in_=g[:])
                else:
                    nc.sync.dma_start(out=emb_out[768:800, :], in_=g[:32, :])
                pt = ps_tr.tile([128, 128], F32, tag="ps_tr")
                nc.tensor.transpose(out=pt[:], in_=g[:], identity=ident[:])
                nc.vector.tensor_copy(out=embT[:, j * 128 : (j + 1) * 128], in_=pt[:])

            # ---- candidate gathers + transposes
            for j in range(CAND_TILES):
                g = gpool.tile([128, EMB], F32, tag="gath")
                nc.gpsimd.indirect_dma_start(
                    out=g[:],
                    out_offset=None,
                    in_=emb_table[:],
                    in_offset=bass.IndirectOffsetOnAxis(ap=cand_s[:, j : j + 1], axis=0),
                )
                nc.sync.dma_start(out=emb_cand_out[j * 128 : (j + 1) * 128, :], in_=g[:])
                pt = ps_tr.tile([128, 128], F32, tag="ps_tr")
                nc.tensor.transpose(out=pt[:], in_=g[:], identity=ident[:])
                nc.scalar.copy(out=embcT[:, j * 128 : (j + 1) * 128], in_=pt[:])

            # ---- session projections: pkbT = Wk1^T (Wk^T embT + bk) + b1
            hk_ps = ps_proj.tile([128, SESS], F32, tag="ps_proj")
            nc.tensor.matmul(out=hk_ps[:, :512], lhsT=Wk_s[:], rhs=embT[:, :512], start=True, stop=True)
            nc.tensor.matmul(out=hk_ps[:, 512:], lhsT=Wk_s[:], rhs=embT[:, 512:SESS], start=True, stop=True)
            hkT = work.tile([128, SESS], F32, tag="hkT")
            nc.scalar.add(out=hkT[:], in_=hk_ps[:], add=bk_s[:, 0:1])
            pk_ps = ps_proj.tile([128, SESS], F32, tag="ps_proj")
            nc.tensor.matmul(out=pk_ps[:, :512], lhsT=Wk1_s[:], rhs=hkT[:, :512], start=True, stop=True)
            nc.tensor.matmul(out=pk_ps[:, 512:], lhsT=Wk1_s[:], rhs=hkT[:, 512:], start=True, stop=True)
            pkbT = const.tile([128, SESS], F32)
            nc.scalar.add(out=pkbT[:], in_=pk_ps[:], add=b1_s[:, 0:1])

            # ---- hidden_demand -> logsumexp over L (values are tiny: direct
            # exp-sum-log is numerically safe here)
            sums = const.tile([128, D * B], F32)
            for j in range(D):
                hd_ps = ps_hd.tile([128, SESS], F32, tag="ps_hd")
                wslice = Wd_s[:, j * 128 : (j + 1) * 128]
                nc.tensor.matmul(out=hd_ps[:, :512], lhsT=wslice, rhs=embT[:, :512], start=True, stop=True)
                nc.tensor.matmul(out=hd_ps[:, 512:], lhsT=wslice, rhs=embT[:, 512:SESS], start=True, stop=True)
                ex = expp.tile([128, SESS], F32, tag="exp")
                nc.scalar.activation(out=ex[:], in_=hd_ps[:], func=AF.Exp)
                nc.vector.tensor_reduce(
                    out=sums[:, j * B : (j + 1) * B],
                    in_=ex[:].rearrange("p (b l) -> p b l", l=L),
                    axis=mybir.AxisListType.X,
                    op=ALU.add,
                )
            aggT = const.tile([128, D * B], F32)  # [h, d*16+b]
            nc.scalar.activation(out=aggT[:], in_=sums[:], func=AF.Ln)
            nc.sync.dma_start(out=aggT_out[:], in_=aggT[:])

            # ---- pdT = Wd1^T @ aggT  [h, d*16+b]
            pd_ps = ps_tr.tile([128, 128], F32, tag="ps_tr")
            nc.tensor.matmul(out=pd_ps[:], lhsT=Wd1_s[:], rhs=aggT[:], start=True, stop=True)
            pdT = const.tile([128, D * B], F32)
            nc.scalar.copy(out=pdT[:], in_=pd_ps[:])

            # ---- candidate projections: U = Wk1^T (Wk^T embcT + bk) + b1
            hkc_ps = ps_proj.tile([128, NC_PER], F32, tag="ps_proj")
            nc.tensor.matmul(out=hkc_ps[:, :512], lhsT=Wk_s[:], rhs=embcT[:, :512], start=True, stop=True)
            nc.tensor.matmul(out=hkc_ps[:, 512:], lhsT=Wk_s[:], rhs=embcT[:, 512:], start=True, stop=True)
            hkcT = work.tile([128, NC_PER], F32, tag="hkcT")
            nc.scalar.add(out=hkcT[:], in_=hkc_ps[:], add=bk_s[:, 0:1])
            pkc_ps = ps_proj.tile([128, NC_PER], F32, tag="ps_proj")
            nc.tensor.matmul(out=pkc_ps[:, :512], lhsT=Wk1_s[:], rhs=hkcT[:, :512], start=True, stop=True)
            nc.tensor.matmul(out=pkc_ps[:, 512:], lhsT=Wk1_s[:], rhs=hkcT[:, 512:], start=True, stop=True)
            u_dt = _SCORE_DTYPES.get(U_DT, F32) if U_DT != "f32r" else F32
            U = const.tile([128, NC_PER], u_dt)
            nc.scalar.add(out=U[:], in_=pkc_ps[:], add=b1_s[:, 0:1])

            # ---- catgy_score^T-ish: rows d*16+b, cols categories
            cat_ps = ps_proj.tile([128, NCAT], F32, tag="ps_proj")
            nc.tensor.matmul(out=cat_ps[:, :512], lhsT=aggT[:], rhs=Wc_s[:, :512], start=True, stop=False)
            nc.tensor.matmul(out=cat_ps[:, :512], lhsT=ones1[:], rhs=bc_s[:, :512], start=False, stop=True)
            nc.tensor.matmul(out=cat_ps[:, 512:], lhsT=aggT[:], rhs=Wc_s[:, 512:], start=True, stop=False)
            nc.tensor.matmul(out=cat_ps[:, 512:], lhsT=ones1[:], rhs=bc_s[:, 512:], start=False, stop=True)
            cat_sb = work.tile([128, NCAT], F32, tag="cat_sb")
            nc.scalar.copy(out=cat_sb[:], in_=cat_ps[:])
            nc.sync.dma_start(out=catgy_out[:], in_=cat_sb[:])

            # ---- demand_score: S[h,(b,d,l)] = relu(pkb[h,(b,l)] + pd[h,(d,b)])
            # (stays f32: only 50 small matmuls, keeps demand_score near-exact)
            mm_dt = _SCORE_DTYPES[SCORE_DT]
            S = const.tile([128, B, D, L], F32)
            pkb_ap = bass.AP(
                tensor=pkbT[:].tensor,
                offset=pkbT[:].offset,
                ap=[pkbT[:].ap[0], [L, B], [0, D], [1, L]],
            )
            pd_ap = bass.AP(
                tensor=pdT[:].tensor,
                offset=pdT[:].offset,
                ap=[pdT[:].ap[0], [1, B], [B, D], [0, L]],
            )
            s_eng = nc.gpsimd if S_ENGINE == "gpsimd" else nc.vector
            s_eng.tensor_tensor(out=S[:], in0=pkb_ap, in1=pd_ap, op=ALU.add)
            s_eng.tensor_scalar_max(S[:], S[:], 0.0)
            if SCORE_DT == "f32":
                wsc_mm = wsc_s
            else:
                wsc_mm = const.tile([H, 1], mm_dt)
                nc.vector.tensor_copy(out=wsc_mm[:], in_=wsc_s[:])
            Sf = S[:].rearrange("p b d l -> p (b d l)")
            t2_ps = ps_tr.tile([128, 64], F32, tag="ps_tr")
            for m in range(50):
                nc.tensor.matmul(
                    out=t2_ps[:, m : m + 1],
                    lhsT=Sf[:, m * 128 : (m + 1) * 128],
                    rhs=wsc_s[:],
                    start=True,
                    stop=True,
                )
            t2_sb = work.tile([128, 64], F32, tag="t2_sb")
            nc.vector.tensor_copy(out=t2_sb[:, :50], in_=t2_ps[:, :50])
            nc.sync.dma_start(out=t2_out[:], in_=t2_sb[:, :50])

            # ---- main loop: per (b,d) pair r, relu-tile V_r = relu(U + pd_r),
            # then 8 matmuls with lhsT = 128-candidate chunk of V_r (weights)
            # and rhs = w_score: each lands a (128,1) column at pall[:, r*8+c].
            # Column order r*8+c lets the first psum half drain at mid-loop.
            pall = ps_dsc.tile([128, CAND_TILES * B * D], F32, tag="dsc")
            dsb = work.tile([128, CAND_TILES * B * D], F32, tag="dsb")
            half = CAND_TILES * B * D // 2
            for r in range(B * D):
                b_, d_ = r // D, r % D
                col = d_ * B + b_
                V = vpool.tile([128, NC_PER], mm_dt, tag="V")
                if (r * VEC_SHARE) % 16 < VEC_SHARE:
                    nc.vector.tensor_scalar(
                        V[:], U[:], pdT[:, col : col + 1], 0.0, ALU.add, ALU.max
                    )
                else:
                    nc.scalar.activation(
                        out=V[:], in_=U[:], func=AF.Relu, bias=pdT[:, col : col + 1]
                    )
                for c in range(CAND_TILES):
                    cr = r * CAND_TILES + c
                    nc.tensor.matmul(
                        out=pall[:, cr : cr + 1],
                        lhsT=V[:, c * 128 : (c + 1) * 128],
                        rhs=wsc_mm[:],
                        start=True, stop=True,
                    )
                if (r + 1) % 32 == 0:
                    q0 = (r + 1 - 32) * CAND_TILES
                    q1 = (r + 1) * CAND_TILES
                    nc.scalar.copy(out=dsb[:, q0:q1], in_=pall[:, q0:q1])
                    nc.sync.dma_start(out=dsc_out[:, q0:q1], in_=dsb[:, q0:q1])


def _build():
    key = (SCORE_DT, U_DT, VEC_SHARE, S_ENGINE)
    if key in _CACHE:
        return _CACHE[key]
    nc = bacc.Bacc("TRN2", target_bir_lowering=False, debug=False, num_devices=NCORES)
    _emit(nc)
    nc.compile()
    _CACHE[key] = nc
    return nc


def kernel(**inputs):
    nc = _build()

    input_ids = np.asarray(inputs["input"]).astype(np.int32)  # (B, L)
    cand = np.asarray(inputs["candidate_pool_category"]).astype(np.int32)  # (NITEMS,)
    emb_table = np.ascontiguousarray(np.asarray(inputs["emb_table"], dtype=np.float32))
    Wd = np.ascontiguousarray(np.asarray(inputs["Wd"], dtype=np.float32))
    Wk = np.ascontiguousarray(np.asarray(inputs["Wk"], dtype=np.float32))
    bk = np.asarray(inputs["bk"], dtype=np.float32).reshape(H, 1)
    W1 = np.asarray(inputs["W1"], dtype=np.float32)
    Wd1 = np.ascontiguousarray(W1[:H])
    Wk1 = np.ascontiguousarray(W1[H:])
    b1 = np.asarray(inputs["b1"], dtype=np.float32).reshape(H, 1)
    wsc = np.asarray(inputs["w_score"], dtype=np.float32).reshape(H, 1)
    Wc = np.ascontiguousarray(np.asarray(inputs["Wc"], dtype=np.float32))
    bc = np.asarray(inputs["bc"], dtype=np.float32).reshape(1, NCAT)

    sess_pad = np.zeros(SESS_TILES * 128, np.int32)
    sess_pad[:SESS] = input_ids.reshape(-1)
    sess_2d = np.ascontiguousarray(sess_pad.reshape(SESS_TILES, 128).T)

    shared = dict(
        emb_table=emb_table, sess_idx=sess_2d, Wk=Wk, Wd=Wd, Wd1=Wd1, Wk1=Wk1,
        Wc=Wc, bk=bk, b1=b1, wsc=wsc, bc=bc,
    )
    in_maps = []
    for c in range(NCORES):
        sl = cand[c * NC_PER : (c + 1) * NC_PER]
        cand_2d = np.ascontiguousarray(sl.reshape(CAND_TILES, 128).T)
        m = dict(shared)
        m["cand_idx"] = cand_2d
        in_maps.append(m)

    trace = bool(int(os.environ.get("K_TRACE", "0")))
    res = run_bass_kernel_spmd(nc, in_maps, list(range(NCORES)), trace=trace)
    kernel.last_results = res

    r0 = res.results[0]
    emb = r0["emb_out"].reshape(B, L, EMB)
    emb_cand = np.concatenate(
        [res.results[c]["emb_cand_out"] for c in range(NCORES)], axis=0
    )
    # per-core dsc_out is (128 n', 1024 cols=(r, c)); decode to (r, n) and
    # stack core slices along n
    dsc_cores = []
    for c in range(NCORES):
        a = res.results[c]["dsc_out"].reshape(128, B * D, CAND_TILES)
        dsc_cores.append(np.transpose(a, (1, 2, 0)).reshape(B * D, NC_PER))
    dsc = np.concatenate(dsc_cores, axis=1).reshape(B, D, NITEMS)

    aggT = r0["aggT_out"]  # [h, d*16+b]
    agg = aggT.T.reshape(D, B, H).transpose(1, 0, 2)  # (B, D, H)
    catgy_score = r0["catgy_out"].reshape(D, B, NCAT).transpose(1, 0, 2)

    # t2_out[c, m] = score of flat (b*D*L + d*L + l) index m*128+c
    t2 = r0["t2_out"]
    t2bdl = np.ascontiguousarray(t2.T).reshape(-1)[: B * D * L].reshape(B, D, L)
    # faithful torch .view(B, D, L) of a contiguous (B, L, D) tensor
    demand_score = t2bdl.transpose(0, 2, 1).reshape(B, D, L)

    hn = agg / (np.linalg.norm(agg, axis=-1, keepdims=True) + 1e-12)
    sim = np.einsum("bdh,beh->bde", hn, hn)
    off = sim * (1.0 - np.eye(D, dtype=sim.dtype))
    demand_sim_loss = np.float32(off.sum() / (B * D * (D - 1)))

    return (catgy_score, demand_score, dsc, emb, emb_cand, demand_sim_loss)
